# revision 4
# baseline (speedup 1.0000x reference)
"""Trainium2 Bass kernel for bidirectional GRU (nn_Bidirectional).

Differences vs baseline:
- Phase A+B fused: xp = x @ Wc + bc with Wc = w_in*diag(s1) @ wx folded on
  host (K=128 instead of two chained matmuls).
- Recurrence: per-step matmuls split by h-halves (k01 then k23) so the PE
  consumes fresh h strips as late as possible and gate psums complete as
  early as possible; xp for z,r gates is identity-matmul-folded into PSUM;
  gate chain uses the u-form blend (only 2 elementwise ops after tanh),
  blend runs on gpsimd to unload DVE.
- y-projection done once per 8-step chunk from an SBUF h-history ring
  (no per-step y matmuls); single y DMA at the end.
"""

import os
import sys
from contextlib import ExitStack

import numpy as np
import ml_dtypes

if "/opt/trn_rl_repo" not in sys.path:
    sys.path.insert(0, "/opt/trn_rl_repo")

WH_FP8 = os.environ.get("WH_FP8", "1") == "1"
FP8S = 16.0  # scale factor applied to wh (and xp z/r part) when WH_FP8

B, T, F, H, O = 64, 512, 128, 512, 8
EPS = 1e-3
NCORES = 8
BC = B // 4          # batch per core = 16
KT = H // 128        # 4 h-strips
MT = 3 * H // 128    # 12 gate strips (z0..3, r0..3, h0..3)
TOK = T * BC         # 8192 tokens per core, time-major: tok = t*BC + b
ABCH = 512           # tokens per phase-AB chunk
NABCH = TOK // ABCH  # 16
SCH = 8              # recurrence steps per chunk
NSCH = T // SCH      # 64 chunks
SB = SCH * BC        # 128 tokens per recurrence chunk
BF16 = ml_dtypes.bfloat16

_cache = {}

# gate m-index layout: z gates m0-3, r gates m4-7, h gates m8-11.
# half hx covers h strips {2hx, 2hx+1}; its psum tile g{hx} slots:
#   0,1 = z strips; 2,3 = r strips; 4,5 = h-gate strips.
G_MS = [[0, 1, 4, 5, 8, 9], [2, 3, 6, 7, 10, 11]]


def _build(has_bh: bool, reps: int = 1):
    import concourse.bass as bass
    import concourse.bacc as bacc
    import concourse.tile as tile
    import concourse.mybir as mybir

    assert not has_bh, "v2 builds only the zero-recurrent-bias variant"

    dt = mybir.dt
    f32 = dt.float32
    bf = dt.bfloat16
    AF = mybir.ActivationFunctionType
    OP = mybir.AluOpType
    ds = bass.ds

    nc = bacc.Bacc("TRN2", target_bir_lowering=False, debug=False,
                   num_devices=NCORES)

    whdt = dt.float8e4 if WH_FP8 else bf
    xT = nc.dram_tensor("xT", [F, TOK], bf, kind="ExternalInput").ap()
    wc = nc.dram_tensor("wc", [128, MT, 128], bf, kind="ExternalInput").ap()
    bc = nc.dram_tensor("bc", [128, MT], f32, kind="ExternalInput").ap()
    wh = nc.dram_tensor("wh", [128, KT, MT, 128], whdt,
                        kind="ExternalInput").ap()
    ident = nc.dram_tensor("ident", [128, 128], bf, kind="ExternalInput").ap()
    wo = nc.dram_tensor("wo", [128, KT, O], bf, kind="ExternalInput").ap()
    bo = nc.dram_tensor("bo", [O, 1], f32, kind="ExternalInput").ap()
    yT = nc.dram_tensor("yT", [O, TOK], f32, kind="ExternalOutput").ap()

    with tile.TileContext(nc) as tc, ExitStack() as ctx:
        consts = ctx.enter_context(tc.tile_pool(name="consts", bufs=1))
        big = ctx.enter_context(tc.tile_pool(name="big", bufs=1))

        # ---------- constants ----------
        wc_sb = consts.tile([128, MT, 128], bf)
        nc.sync.dma_start(out=wc_sb, in_=wc)
        bc_sb = consts.tile([128, MT], f32)
        nc.sync.dma_start(out=bc_sb, in_=bc)
        wh_sb = consts.tile([128, KT, MT, 128], whdt)
        nc.sync.dma_start(out=wh_sb, in_=wh)
        id_sb = consts.tile([128, 128], bf)
        nc.sync.dma_start(out=id_sb, in_=ident)
        wo_sb = consts.tile([128, KT, O], bf)
        nc.sync.dma_start(out=wo_sb, in_=wo)
        bo_sb = consts.tile([O, 1], f32)
        nc.sync.dma_start(out=bo_sb, in_=bo)

        xt_sb = big.tile([128, TOK], bf, tag="xt")
        nc.sync.dma_start(out=xt_sb, in_=xT)

        dram = ctx.enter_context(tc.tile_pool(name="dram", bufs=1,
                                              space="DRAM"))
        xp_dr = dram.tile([128, MT, TOK + 2 * SB], bf)

        def phase_ab():
            # ---------- phase AB: xp = x @ Wc + bc -> DRAM bf16 ----------
            # bufs=4 when run standalone (before the loop pools allocate);
            # inside the reps timing loop only 2 banks remain free.
            ab_bufs = 4 if reps == 1 else 2
            with tc.tile_pool(name="psab", bufs=ab_bufs,
                              space="PSUM") as psab, \
                    tc.tile_pool(name="stab", bufs=6) as stab:
                for c in range(NABCH):
                    sl = slice(ABCH * c, ABCH * (c + 1))
                    for m in range(MT):
                        ps = psab.tile([128, ABCH], f32, tag="ab")
                        nc.tensor.matmul(ps, wc_sb[:, m, :], xt_sb[:, sl],
                                         start=True, stop=True)
                        st = stab.tile([128, ABCH], bf, tag="st")
                        if m % 3 == 0:
                            nc.scalar.activation(st, ps, AF.Identity,
                                                 bias=bc_sb[:, m:m + 1],
                                                 scale=1.0)
                        else:
                            nc.vector.tensor_scalar_add(st, ps,
                                                        bc_sb[:, m:m + 1])
                        nc.sync.dma_start(out=xp_dr[:, m, sl], in_=st)

        if reps == 1:
            phase_ab()

        # ---------- phase C: recurrence ----------
        ybuf = big.tile([O, TOK], f32, tag="ybuf")
        histA = big.tile([128, KT, SCH, BC], bf, tag="histA")
        histB = big.tile([128, KT, SCH, BC], bf, tag="histB")

        xpA = big.tile([128, MT, SB], bf, tag="xpA")
        xpB = big.tile([128, MT, SB], bf, tag="xpB")

        psG = ctx.enter_context(tc.tile_pool(name="psG", bufs=2,
                                             space="PSUM"))
        psY = ctx.enter_context(tc.tile_pool(name="psY", bufs=2,
                                             space="PSUM"))
        gates = ctx.enter_context(tc.tile_pool(name="gates", bufs=2))

        def step(xc, j, hist, histo, gts, sgs):
            """One GRU step j (0..7) of a chunk."""
            def h_in(k):
                if j == 0:
                    return histo[:, k, SCH - 1, :]
                return hist[:, k, j - 1, :]

            tsl = slice(BC * j, BC * (j + 1))

            # --- alpha: id-folds + all k01 matmuls ---
            # One accumulation group per g-tile (bank): start=True zeroes
            # the whole bank on the first id matmul; every later matmul
            # start=False (adds onto zeros where nothing was written yet);
            # stop=True on the final k=3 matmul of the bank.
            for hx in (0, 1):
                g = gts[hx]
                # xp for z,r strips of this half: m pairs {2hx,2hx+1} and
                # {4+2hx,5+2hx}; two id matmuls (contiguous m pairs)
                src = xc[:, :, tsl]
                nc.tensor.matmul(g[:, 0:2, :], id_sb,
                                 src[:, 2 * hx:2 * hx + 2, :],
                                 start=True, stop=False)
                nc.tensor.matmul(g[:, 2:4, :], id_sb,
                                 src[:, 4 + 2 * hx:6 + 2 * hx, :],
                                 start=False, stop=False)
            for hx in (0, 1):
                g = gts[hx]
                for slot, m in enumerate(G_MS[hx]):
                    for k in (0, 1):
                        nc.tensor.matmul(g[:, slot, :], wh_sb[:, k, m, :],
                                         h_in(k), start=False, stop=False)

            # --- beta: k23, half0 first, then chain per half ---
            for hx in (0, 1):
                g = gts[hx]
                for slot, m in enumerate(G_MS[hx]):
                    for k in (2, 3):
                        nc.tensor.matmul(g[:, slot, :], wh_sb[:, k, m, :],
                                         h_in(k), start=False,
                                         stop=(k == 3 and slot == 5))
                # sigmoid over the whole tile (h-part result unused, but the
                # full-tile read keeps ACT off the bank until PE finished it)
                s = sgs[hx]
                nc.scalar.activation(s, g, AF.Sigmoid,
                                     scale=(1.0 / FP8S) if WH_FP8 else 1.0)
                u = gates.tile([128, 2, BC], bf, tag=f"u{hx}")
                h_in2 = (histo[:, 2 * hx:2 * hx + 2, SCH - 1, :] if j == 0
                         else hist[:, 2 * hx:2 * hx + 2, j - 1, :])
                nc.gpsimd.tensor_mul(u, s[:, 0:2, :], h_in2)
                om = gates.tile([128, 2, BC], bf, tag=f"om{hx}")
                nc.gpsimd.tensor_sub(om, ones_sb, s[:, 0:2, :])
                t1 = gates.tile([128, 2, BC], bf, tag=f"t1{hx}")
                if WH_FP8:
                    nc.vector.scalar_tensor_tensor(
                        t1, g[:, 4:6, :], 1.0 / FP8S, s[:, 2:4, :],
                        OP.mult, OP.mult)
                else:
                    nc.vector.tensor_mul(t1, g[:, 4:6, :], s[:, 2:4, :])
                t2 = gates.tile([128, 2, BC], bf, tag=f"t2{hx}")
                nc.vector.tensor_add(t2, t1,
                                     xc[:, 8 + 2 * hx:10 + 2 * hx, tsl])
                hh = gates.tile([128, 2, BC], bf, tag=f"hh{hx}")
                nc.scalar.activation(hh, t2, AF.Tanh)
                gg = gates.tile([128, 2, BC], bf, tag=f"gg{hx}")
                nc.gpsimd.tensor_mul(gg, om, hh)
                nc.gpsimd.tensor_add(hist[:, 2 * hx:2 * hx + 2, j, :],
                                     gg, u)

        ones_sb = consts.tile([128, 2, BC], bf)
        nc.vector.memset(ones_sb, 1.0)

        def chunk(ci_ap, xc, hist, histo, coff):
            """Run SCH steps + y-projection for one chunk. ci_ap = dynamic
            token offset AP expr (ds), coff python-side parity only."""
            for j in range(SCH):
                gts = [psG.tile([128, 6, BC], f32, tag=f"g{hx}",
                                name=f"g{hx}")
                       for hx in (0, 1)]
                sgs = [gates.tile([128, 6, BC], bf, tag=f"s{hx}",
                                  name=f"s{hx}")
                       for hx in (0, 1)]
                step(xc, j, hist, histo, gts, sgs)
            psy = psY.tile([O, SB], f32, tag="y")
            hflat = hist.rearrange("p k j t -> p k (j t)")
            for k in range(KT):
                nc.tensor.matmul(psy, wo_sb[:, k, :], hflat[:, k, :],
                                 start=(k == 0), stop=(k == KT - 1))
            nc.scalar.activation(ybuf[:, ci_ap], psy, AF.Identity,
                                 bias=bo_sb, scale=1.0)

        def phase_c():
            nc.vector.memset(histB[:, :, SCH - 1, :], 0.0)
            nc.sync.dma_start(out=xpA, in_=xp_dr[:, :, 0:SB])
            with tc.For_i(0, NSCH, 2,
                          hint_engines=(mybir.EngineType.PE,)) as i:
                nc.sync.dma_start(out=xpB,
                                  in_=xp_dr[:, :, ds((i + 1) * SB, SB)])
                chunk(ds(i * SB, SB), xpA, histA, histB, 0)
                nc.sync.dma_start(out=xpA,
                                  in_=xp_dr[:, :, ds((i + 2) * SB, SB)])
                chunk(ds((i + 1) * SB, SB), xpB, histB, histA, 1)

        if reps == 1:
            phase_c()
        else:
            with tc.For_i(0, reps, 1):
                phase_ab()
                phase_c()

        nc.sync.dma_start(out=yT, in_=ybuf)

    nc.compile()
    return nc


def _get_program(has_bh: bool):
    key = ("prog", has_bh)
    if key not in _cache:
        _cache[key] = _build(has_bh)
    return _cache[key]


def _prep_core(x_shard, rev, w_in, s1, b1, wx, wh, bb, wo_half, bias_out):
    """Per-core input map (numpy, device layouts/dtypes)."""
    xs = x_shard[:, ::-1] if rev else x_shard          # [BC, T, F]
    xTc = np.ascontiguousarray(xs.transpose(2, 1, 0)).reshape(F, TOK)
    win_s = w_in * s1[None, :]                          # [F, H]
    Wc = win_s.astype(np.float32) @ wx                  # [F, 3H]
    bias_xp = np.concatenate([bb[0, :2 * H] + bb[1, :2 * H], bb[0, 2 * H:]])
    bcv = b1 @ wx + bias_xp                             # [3H]
    wh_dev = np.ascontiguousarray(
        wh.reshape(KT, 128, MT, 128).transpose(1, 0, 2, 3))
    if WH_FP8:
        # scale weights into e4m3 normal range; z/r psums carry the x16
        # which the chain divides back out. xp for z/r gates must carry the
        # same x16: fold it into wc/bc columns for gate strips m<8.
        Wc = Wc.copy()
        Wc[:, :2 * H] *= FP8S
        bcv = bcv.copy()
        bcv[:2 * H] *= FP8S
        wh_dev = (wh_dev * FP8S).astype(ml_dtypes.float8_e4m3)
    else:
        wh_dev = wh_dev.astype(BF16)
    return {
        "xT": xTc.astype(BF16),
        "wc": np.ascontiguousarray(
            Wc.reshape(F, MT, 128)).astype(BF16),
        "bc": np.ascontiguousarray(
            bcv.reshape(MT, 128).T.astype(np.float32)),
        "wh": wh_dev,
        "ident": np.eye(128).astype(BF16),
        "wo": np.ascontiguousarray(
            wo_half.reshape(KT, 128, O).transpose(1, 0, 2)).astype(BF16),
        "bo": bias_out.reshape(O, 1).astype(np.float32),
    }


def kernel(x, w_in, b_in, g1, be1, m1, v1, wxf, whf, bf, wxb, whb, bb,
           w_out, b_out, g2, be2, m2, v2):
    from concourse.bass_utils import run_bass_kernel_spmd

    args = locals()
    np_in = {k: np.asarray(args[k], np.float32) for k in (
        "x", "w_in", "b_in", "g1", "be1", "m1", "v1", "wxf", "whf", "bf",
        "wxb", "whb", "bb", "w_out", "b_out", "g2", "be2", "m2", "v2")}

    s1 = np_in["g1"] / np.sqrt(np_in["v1"] + EPS)
    b1 = (np_in["b_in"] - np_in["m1"]) * s1 + np_in["be1"]
    s2 = np_in["g2"] / np.sqrt(np_in["v2"] + EPS)
    b2 = (np_in["b_out"] - np_in["m2"]) * s2 + np_in["be2"]
    Ws = np_in["w_out"] * s2[None, :]

    nc = _get_program(False)

    in_maps = []
    for c in range(NCORES):
        d, s = c // 4, c % 4
        shard = np_in["x"][BC * s:BC * (s + 1)]
        if d == 0:
            m = _prep_core(shard, False, np_in["w_in"], s1, b1,
                           np_in["wxf"], np_in["whf"], np_in["bf"],
                           Ws[:H], b2)
        else:
            m = _prep_core(shard, True, np_in["w_in"], s1, b1,
                           np_in["wxb"], np_in["whb"], np_in["bb"],
                           Ws[H:], np.zeros(O, np.float32))
        in_maps.append(m)

    res = run_bass_kernel_spmd(nc, in_maps, core_ids=list(range(NCORES)))
    outs = res.results

    y = np.zeros((B, T, O), np.float32)
    for s in range(4):
        yf = outs[s]["yT"].reshape(O, T, BC)
        yb = outs[4 + s]["yT"].reshape(O, T, BC)[:, ::-1]
        y[BC * s:BC * (s + 1)] = (yf + yb).transpose(2, 1, 0)
    return y
